# revision 2
# baseline (speedup 1.0000x reference)
"""MoE layer (8 experts, top-2) as an F-sliced expert-parallel Trainium2
Bass kernel.

Strategy (v2 — balanced F-slicing):
  - Host: gating matmul (tiny), top-2 routing, gather tokens per expert into
    one concatenated stream (each expert's block padded to a multiple of
    128).  TP = padded stream length (~mean load * 8, not max load * 8).
  - Device (8 NeuronCores, SPMD): core c owns columns [c*512, (c+1)*512) of
    D_FF for EVERY expert (W1/W2/b1 slices of all 8 experts are SBUF
    resident, 16 MiB total).  Every core processes the WHOLE token stream:
    per group of <=512 tokens (single expert per group), mm1 (4 f-tiles x 8
    accumulating matmuls) -> relu+bias on ScalarE into transient bf16 h
    tiles -> mm2 (per 128-token m-tile, 2 d-halves, 4 accumulating matmuls)
    -> combine-weight scale on VectorE -> bf16 partial y to DRAM.
  - Host: sum the 8 partial y streams (f32), scatter-add the two expert
    contributions per token, add b2 term.

  This balances load perfectly across cores (every core does TP tokens *
  1/8 of the FFN) at the cost of 8x duplicated x reads and 8 partial y
  writes — ~83 MiB DMA/core, well under the ~437 us of bf16 PE time.

Layouts (device side, per core c; fs = slice(c*512, (c+1)*512)):
  xT : [128, KO1*TP]  bf16  xT[p, ko*TP + t]          = x_stream[t, ko*128+p]
  w1 : [128, E*4*8*128] bf16 w1[p, ((e*4+fl)*8+ko)*128+f'] = W1[e, ko*128+p, c*512+fl*128+f']
  w2 : [128, E*4*1024] bf16 w2[p, (e*4+fl)*1024 + d]  = W2[e, c*512+fl*128+p, d]
  b1 : [128, E*4]     f32   b1[p, e*4+fl]             = b1_vec[e, c*512+fl*128+p]
  cw : [128, TP/128]  f32   cw[p, o]                  = combine_weight[o*128+p]
  y  : [128, (TP/128)*1024] bf16 (out) y[p, o*D+d]    = y_partial[o*128+p, d]
"""

import os

import numpy as np
import ml_dtypes

D_MODEL = 1024
D_FF = 4096
N_EXPERTS = 8
TOP_K = 2
B, S = 4, 2048
T = B * S
P = 128
KO1 = D_MODEL // P    # 8  k-subtiles for matmul1
FL = 4                # local f-tiles (512 / 128)
FSL = D_FF // 8       # 512-wide f-slice per core
N_CORES = 8

BF16 = ml_dtypes.bfloat16

# Compiled-module cache keyed by the per-expert m-tile count tuple.
_NC_CACHE = {}
LAST_RESULTS = None  # BassKernelResults of the most recent run (for test.py)
LAST_IN_MAPS = None  # per-core input maps of the most recent run
LAST_C = None        # spec tuple of the most recent run


def _groups_for_spec(spec):
    """Group list [(expert, global_m_offset, m_tiles)] with a small lead
    group (fast ramp) and a small global tail group (short drain)."""
    groups = []
    moff = 0
    first = True
    for e, me in enumerate(spec):
        sizes = []
        rem = me
        if first and rem >= 3:
            sizes.append(2)
            rem -= 2
        first = False
        last_expert = all(m == 0 for m in spec[e + 1:])
        while rem > 0:
            if last_expert and rem == 4:
                sizes.extend([3, 1])
                rem = 0
            else:
                take = min(4, rem)
                sizes.append(take)
                rem -= take
        local = 0
        for mt in sizes:
            groups.append((e, moff + local, mt))
            local += mt
        moff += me
    return groups


def _build_nc(spec, reps=1):
    import concourse.bass as bass  # noqa: F401
    import concourse.tile as tile
    from concourse import bacc, mybir
    from contextlib import ExitStack

    spec = tuple(spec)
    groups = _groups_for_spec(spec)
    NMT = sum(spec)          # total m-tiles in the stream
    TP = NMT * P             # padded stream length
    NG = len(groups)

    nc = bacc.Bacc("TRN2", target_bir_lowering=False, debug=False,
                   num_devices=N_CORES)

    xT = nc.dram_tensor("xT", [P, KO1 * TP], mybir.dt.bfloat16,
                        kind="ExternalInput")
    w1 = nc.dram_tensor("w1", [P, N_EXPERTS * FL * KO1 * P],
                        mybir.dt.bfloat16, kind="ExternalInput")
    w2 = nc.dram_tensor("w2", [P, N_EXPERTS * FL * D_MODEL],
                        mybir.dt.bfloat16, kind="ExternalInput")
    b1 = nc.dram_tensor("b1", [P, N_EXPERTS * FL], mybir.dt.float32,
                        kind="ExternalInput")
    cw = nc.dram_tensor("cw", [P, NMT], mybir.dt.float32,
                        kind="ExternalInput")
    y = nc.dram_tensor("y", [P, NMT * D_MODEL], mybir.dt.bfloat16,
                       kind="ExternalOutput")

    xT_ap = xT.ap().rearrange("p (ko t) -> p ko t", ko=KO1)
    w1_ap = w1.ap().rearrange("p (e fl ko f) -> p e fl ko f",
                              e=N_EXPERTS, fl=FL, ko=KO1)
    w2_ap = w2.ap().rearrange("p (e fl d) -> p e fl d", e=N_EXPERTS, fl=FL)
    y_ap = y.ap()

    with tile.TileContext(nc) as tc, ExitStack() as ctx:
        wpool = ctx.enter_context(tc.tile_pool(name="wpool", bufs=1))
        xpool = ctx.enter_context(tc.tile_pool(name="xpool", bufs=3))
        hpool = ctx.enter_context(tc.tile_pool(name="hpool", bufs=1))
        ypool = ctx.enter_context(tc.tile_pool(name="ypool", bufs=3))
        ps1 = ctx.enter_context(tc.tile_pool(name="ps1", bufs=2, space="PSUM"))
        ps2 = ctx.enter_context(tc.tile_pool(name="ps2", bufs=5, space="PSUM"))

        for rep in range(reps):
            b1s = wpool.tile([P, N_EXPERTS * FL], mybir.dt.float32,
                             tag="b1s", name="b1s")
            cws = wpool.tile([P, NMT], mybir.dt.float32, tag="cws",
                             name="cws")
            w1s = wpool.tile([P, N_EXPERTS, FL, KO1, P], mybir.dt.bfloat16,
                             tag="w1s", name="w1s")
            w2s = wpool.tile([P, N_EXPERTS, FL, D_MODEL], mybir.dt.bfloat16,
                             tag="w2s", name="w2s")

            def load_x(gi):
                e, moff, mt = groups[gi]
                xt = xpool.tile([P, KO1, 512], mybir.dt.bfloat16, tag="xg",
                                name=f"x_{gi}")
                c0 = moff * P
                nc.gpsimd.dma_start(xt[:, :, :mt * P],
                                    xT_ap[:, :, c0:c0 + mt * P])
                return xt

            # Ramp-critical DMAs first: w1 slice of the first expert in 4
            # per-fl chunks on sync (first mm1 chain starts after chunk 0),
            # x group 0 + b1 on gpsimd.
            e0 = groups[0][0]
            for fl in range(FL):
                nc.sync.dma_start(w1s[:, e0, fl], w1_ap[:, e0, fl])
            x_pending = {0: load_x(0)}
            nc.gpsimd.dma_start(b1s[:], b1.ap())
            x_pending[1] = load_x(1) if NG > 1 else None
            nc.gpsimd.dma_start(cws[:], cw.ap())

            # Remaining weights in first-use order, alternating queues.
            # w2[e0] is needed one group after the start; w1[e] before
            # its first mm1 group.
            used = sorted(set(e for e, _, _ in groups))
            seq = [("w2", e0)]
            for e in used:
                if e == e0:
                    continue
                seq.append(("w1", e))
                seq.append(("w2", e))
            for i, (kind, e) in enumerate(seq):
                eng = nc.sync if i % 2 == 0 else nc.gpsimd
                if kind == "w1":
                    eng.dma_start(w1s[:, e], w1_ap[:, e])
                else:
                    eng.dma_start(w2s[:, e], w2_ap[:, e])

            def h_tile(fl, par):
                return hpool.tile([P, 512], mybir.dt.bfloat16,
                                  tag=f"h_{fl}_{par}", name=f"h_{fl}_{par}")

            def emit_mm1(gi, fl, xt, h_cur):
                e, moff, mt = groups[gi]
                gtb = mt * P
                pt = ps1.tile([P, 512], mybir.dt.float32, tag="ps1",
                              name="pt1")
                for ko in range(KO1):
                    nc.tensor.matmul(
                        pt[:, :gtb],
                        w1s[:, e, fl, ko, :],
                        xt[:, ko, :gtb],
                        start=(ko == 0),
                        stop=(ko == KO1 - 1),
                    )
                hf = h_tile(fl, gi % 2)
                nc.scalar.activation(
                    hf[:, :gtb], pt[:, :gtb],
                    mybir.ActivationFunctionType.Relu,
                    bias=b1s[:, e * FL + fl:e * FL + fl + 1],
                )
                h_cur.append(hf)

            def emit_chain(gi, m, half, h_prev, ytile):
                e, moff, mt = groups[gi]
                pt2 = ps2.tile([P, 512], mybir.dt.float32, tag="ps2",
                               name="pt2")
                for fl in range(FL):
                    nc.tensor.matmul(
                        pt2[:],
                        h_prev[fl][:, m * P:(m + 1) * P],
                        w2s[:, e, fl, half * 512:(half + 1) * 512],
                        start=(fl == 0),
                        stop=(fl == FL - 1),
                    )
                nc.vector.tensor_scalar_mul(
                    ytile[:, m * D_MODEL + half * 512:
                          m * D_MODEL + (half + 1) * 512],
                    pt2[:],
                    cws[:, moff + m:moff + m + 1],
                )

            h_prev = None
            for gi in range(NG + 1):
                cur = gi if gi < NG else None
                prev = gi - 1 if gi > 0 else None

                if cur is not None and gi + 2 < NG:
                    x_pending[gi + 2] = load_x(gi + 2)

                if prev is not None:
                    pe, pmoff, pmt = groups[prev]
                    chains = [(m, h) for m in range(pmt) for h in range(2)]
                else:
                    chains = []

                if cur is None:
                    # Final drain: per-chain psum -> scale -> dma so the
                    # writeback overlaps the remaining chains' matmuls.
                    for m, half in chains:
                        pt2 = ps2.tile([P, 512], mybir.dt.float32,
                                       tag="ps2", name="pt2")
                        for fl in range(FL):
                            nc.tensor.matmul(
                                pt2[:],
                                h_prev[fl][:, m * P:(m + 1) * P],
                                w2s[:, pe, fl, half * 512:(half + 1) * 512],
                                start=(fl == 0),
                                stop=(fl == FL - 1),
                            )
                        yt = ypool.tile([P, 512], mybir.dt.bfloat16,
                                        tag="ytail", name="ytail")
                        nc.vector.tensor_scalar_mul(
                            yt[:], pt2[:],
                            cws[:, pmoff + m:pmoff + m + 1],
                        )
                        nc.sync.dma_start(
                            y_ap[:, (pmoff + m) * D_MODEL + half * 512:
                                 (pmoff + m) * D_MODEL + (half + 1) * 512],
                            yt[:],
                        )
                    continue

                e, moff, mt = groups[cur]
                xt = x_pending.pop(cur)
                h_cur = []
                ytile = None
                if prev is not None:
                    ytile = ypool.tile([P, pmt * D_MODEL],
                                       mybir.dt.bfloat16, tag="yt",
                                       name="yt")
                    # Distribute prev's 2*pmt mm2 chains over the 4 fl
                    # iterations of cur's mm1.
                    per = [chains[(len(chains) * fl) // FL:
                                  (len(chains) * (fl + 1)) // FL]
                           for fl in range(FL)]
                else:
                    per = [[] for _ in range(FL)]

                for fl in range(FL):
                    emit_mm1(cur, fl, xt, h_cur)
                    for m, half in per[fl]:
                        emit_chain(prev, m, half, h_prev, ytile)
                if prev is not None:
                    nc.sync.dma_start(
                        y_ap[:, pmoff * D_MODEL:(pmoff + pmt) * D_MODEL],
                        ytile[:, :pmt * D_MODEL],
                    )
                h_prev = h_cur

    nc.compile()
    return nc


def _route(x_flat, Wg, bg):
    logits = x_flat.astype(np.float32) @ Wg.astype(np.float32) + bg
    idx = np.argsort(-logits, axis=1, kind="stable")[:, :TOP_K]
    gates = np.take_along_axis(logits, idx, axis=1)  # [T, 2] descending
    e1 = np.exp(gates[:, 1] - gates[:, 0])
    denom = 1.0 + e1
    w = np.stack([1.0 / denom, e1 / denom], axis=1).astype(np.float32)
    return idx.astype(np.int32), w


def _emulate_run(in_maps, spec):
    """Numpy emulation of the device kernel (for layout validation)."""
    NMT = sum(spec)
    TP = NMT * P
    results = []
    for m in in_maps:
        xT = np.asarray(m["xT"], np.float32).reshape(P, KO1, TP)
        w1 = np.asarray(m["w1"], np.float32).reshape(P, N_EXPERTS, FL,
                                                     KO1, P)
        w2 = np.asarray(m["w2"], np.float32).reshape(P, N_EXPERTS, FL,
                                                     D_MODEL)
        b1 = np.asarray(m["b1"], np.float32).reshape(P, N_EXPERTS, FL)
        cwt = np.asarray(m["cw"], np.float32)  # [P, NMT]
        xs = np.ascontiguousarray(xT.transpose(2, 1, 0)).reshape(TP,
                                                                 D_MODEL)
        cwf = np.ascontiguousarray(cwt.T).reshape(TP)
        yv = np.zeros((TP, D_MODEL), np.float32)
        off = 0
        for e, me in enumerate(spec):
            n = me * P
            seg = slice(off, off + n)
            W1e = np.ascontiguousarray(
                w1[:, e].transpose(2, 0, 1, 3)).reshape(D_MODEL, FSL)
            W2e = np.ascontiguousarray(
                w2[:, e].transpose(1, 0, 2)).reshape(FSL, D_MODEL)
            b1e = np.ascontiguousarray(b1[:, e].T).reshape(FSL)
            h = np.maximum(xs[seg] @ W1e + b1e, 0.0)
            h = h.astype(BF16).astype(np.float32)
            yv[seg] = (h @ W2e) * cwf[seg, None]
            off += n
        y_dev = np.ascontiguousarray(
            yv.reshape(NMT, P, D_MODEL).transpose(1, 0, 2)
        ).reshape(P, NMT * D_MODEL).astype(BF16)
        results.append({"y": y_dev})

    class Res:
        pass

    r = Res()
    r.results = results
    r.exec_time_ns = None
    return r


def kernel(x, Wg, bg, W1, b1, W2, b2):
    global LAST_RESULTS, LAST_IN_MAPS, LAST_C
    x = np.asarray(x, dtype=np.float32)
    Wg = np.asarray(Wg, dtype=np.float32)
    bg = np.asarray(bg, dtype=np.float32)
    W1 = np.asarray(W1, dtype=np.float32)
    b1 = np.asarray(b1, dtype=np.float32)
    W2 = np.asarray(W2, dtype=np.float32)
    b2 = np.asarray(b2, dtype=np.float32)

    x_flat = x.reshape(T, D_MODEL)
    idx, w = _route(x_flat, Wg, bg)

    # Per-expert token lists; stream offset + slot map (position of each
    # (token, k) pair inside the padded concatenated stream).
    tok_lists = []
    spec = []
    offs = []
    off = 0
    slot = np.empty((T, TOP_K), dtype=np.int64)
    for e in range(N_EXPERTS):
        mask = (idx[:, 0] == e) | (idx[:, 1] == e)
        tok = np.nonzero(mask)[0]
        tok_lists.append(tok)
        me = (len(tok) + P - 1) // P
        spec.append(me)
        offs.append(off)
        which = (idx[tok, 1] == e).astype(np.int64)  # 0 if k=0 slot else 1
        slot[tok, which] = off + np.arange(len(tok))
        off += me * P
    spec = tuple(spec)
    TP = off
    NMT = TP // P

    # Build the shared token stream + per-core weight slices.
    xs = np.zeros((TP, D_MODEL), dtype=np.float32)
    cw_stream = np.zeros((TP,), dtype=np.float32)
    for e in range(N_EXPERTS):
        tok = tok_lists[e]
        n = len(tok)
        o = offs[e]
        xs[o:o + n] = x_flat[tok]
        cw_stream[o:o + n] = np.where(idx[tok, 0] == e, w[tok, 0],
                                      w[tok, 1])

    xT_dev = np.ascontiguousarray(
        xs.reshape(TP, KO1, P).transpose(2, 1, 0)
    ).reshape(P, KO1 * TP).astype(BF16)
    cw_dev = np.ascontiguousarray(
        cw_stream.reshape(NMT, P).T).astype(np.float32)

    in_maps = []
    for c in range(N_CORES):
        fs = slice(c * FSL, (c + 1) * FSL)
        w1c = np.ascontiguousarray(
            W1[:, :, fs].reshape(N_EXPERTS, KO1, P, FL, P)
            .transpose(2, 0, 3, 1, 4)
        ).reshape(P, N_EXPERTS * FL * KO1 * P).astype(BF16)
        w2c = np.ascontiguousarray(
            W2[:, fs, :].reshape(N_EXPERTS, FL, P, D_MODEL)
            .transpose(2, 0, 1, 3)
        ).reshape(P, N_EXPERTS * FL * D_MODEL).astype(BF16)
        b1c = np.ascontiguousarray(
            b1[:, fs].reshape(N_EXPERTS, FL, P).transpose(2, 0, 1)
        ).reshape(P, N_EXPERTS * FL).astype(np.float32)
        in_maps.append({
            "xT": xT_dev,
            "w1": w1c,
            "w2": w2c,
            "b1": b1c,
            "cw": cw_dev,
        })

    LAST_IN_MAPS = in_maps
    LAST_C = spec

    if os.environ.get("MOE_KERNEL_EMULATE", "0") == "1":
        res = _emulate_run(in_maps, spec)
    else:
        if spec not in _NC_CACHE:
            _NC_CACHE[spec] = _build_nc(spec)
        nc = _NC_CACHE[spec]

        from concourse.bass_utils import run_bass_kernel_spmd

        trace = os.environ.get("MOE_KERNEL_TRACE", "0") == "1"
        res = run_bass_kernel_spmd(
            nc, in_maps, core_ids=list(range(N_CORES)),
            trace=trace, trace_cores=[0] if trace else None,
        )
    LAST_RESULTS = res

    # Sum the 8 partial y streams, then combine per token.
    Ysum = np.zeros((TP, D_MODEL), dtype=np.float32)
    for c in range(N_CORES):
        y_dev = np.asarray(res.results[c]["y"], dtype=np.float32)
        Ysum += y_dev.reshape(P, NMT, D_MODEL).transpose(1, 0, 2).reshape(
            TP, D_MODEL)

    out_flat = Ysum[slot[:, 0]] + Ysum[slot[:, 1]]

    if np.any(b2):
        out_flat += w[:, 0:1] * b2[idx[:, 0]] + w[:, 1:2] * b2[idx[:, 1]]

    return out_flat.reshape(B, S, D_MODEL).astype(np.float32)


# revision 7
# speedup vs baseline: 1.0778x; 1.0778x over previous
"""MoE layer (8 experts, top-2) as an F-sliced expert-parallel Trainium2
Bass kernel.

Strategy (v2 — balanced F-slicing):
  - Host: gating matmul (tiny), top-2 routing, gather tokens per expert into
    one concatenated stream (each expert's block padded to a multiple of
    128).  TP = padded stream length (~mean load * 8, not max load * 8).
  - Device (8 NeuronCores, SPMD): core c owns columns [c*512, (c+1)*512) of
    D_FF for EVERY expert (W1/W2/b1 slices of all 8 experts are SBUF
    resident, 16 MiB total).  Every core processes the WHOLE token stream:
    per group of <=512 tokens (single expert per group), mm1 (4 f-tiles x 8
    accumulating matmuls) -> relu+bias on ScalarE into transient bf16 h
    tiles -> mm2 (per 128-token m-tile, 2 d-halves, 4 accumulating matmuls)
    -> combine-weight scale on VectorE -> bf16 partial y to DRAM.
  - Host: sum the 8 partial y streams (f32), scatter-add the two expert
    contributions per token, add b2 term.

  This balances load perfectly across cores (every core does TP tokens *
  1/8 of the FFN) at the cost of 8x duplicated x reads and 8 partial y
  writes — ~83 MiB DMA/core, well under the ~437 us of bf16 PE time.

Layouts (device side, per core c; fs = slice(c*512, (c+1)*512)):
  xT : [128, KO1*TP]  bf16  xT[p, ko*TP + t]          = x_stream[t, ko*128+p]
  w1 : [128, E*4*8*128] bf16 w1[p, ((e*4+fl)*8+ko)*128+f'] = W1[e, ko*128+p, c*512+fl*128+f']
  w2 : [128, E*4*1024] bf16 w2[p, (e*4+fl)*1024 + d]  = W2[e, c*512+fl*128+p, d]
  b1 : [128, E*4]     f32   b1[p, e*4+fl]             = b1_vec[e, c*512+fl*128+p]
  cw : [128, TP/128]  f32   cw[p, o]                  = combine_weight[o*128+p]
  y  : [128, (TP/128)*1024] bf16 (out) y[p, o*D+d]    = y_partial[o*128+p, d]
"""

import os

import numpy as np
import ml_dtypes

D_MODEL = 1024
D_FF = 4096
N_EXPERTS = 8
TOP_K = 2
B, S = 4, 2048
T = B * S
P = 128
KO1 = D_MODEL // P    # 8  k-subtiles for matmul1
FL = 4                # local f-tiles (512 / 128)
FSL = D_FF // 8       # 512-wide f-slice per core
N_CORES = 8

BF16 = ml_dtypes.bfloat16

# Compiled-module cache keyed by the per-expert m-tile count tuple.
_NC_CACHE = {}
LAST_RESULTS = None  # BassKernelResults of the most recent run (for test.py)
LAST_IN_MAPS = None  # per-core input maps of the most recent run
LAST_C = None        # spec tuple of the most recent run


def _groups_for_spec(spec):
    """Group list [(expert, global_m_offset, m_tiles)] with a small lead
    group (fast ramp) and a small global tail group (short drain)."""
    groups = []
    moff = 0
    first = True
    for e, me in enumerate(spec):
        sizes = []
        rem = me
        if first and rem >= 3:
            sizes.append(2)
            rem -= 2
        first = False
        last_expert = all(m == 0 for m in spec[e + 1:])
        while rem > 0:
            if last_expert and rem == 4:
                sizes.extend([3, 1])
                rem = 0
            else:
                take = min(4, rem)
                sizes.append(take)
                rem -= take
        local = 0
        for mt in sizes:
            groups.append((e, moff + local, mt))
            local += mt
        moff += me
    return groups


def _build_nc(spec, reps=1):
    import concourse.bass as bass  # noqa: F401
    import concourse.tile as tile
    from concourse import bacc, mybir
    from contextlib import ExitStack

    spec = tuple(spec)
    groups = _groups_for_spec(spec)
    NMT = sum(spec)          # total m-tiles in the stream
    TP = NMT * P             # padded stream length
    NG = len(groups)

    nc = bacc.Bacc("TRN2", target_bir_lowering=False, debug=False,
                   num_devices=N_CORES)

    xT = nc.dram_tensor("xT", [P, KO1 * TP], mybir.dt.bfloat16,
                        kind="ExternalInput")
    w1 = nc.dram_tensor("w1", [P, N_EXPERTS * FL * KO1 * P],
                        mybir.dt.bfloat16, kind="ExternalInput")
    w2 = nc.dram_tensor("w2", [P, N_EXPERTS * FL * D_MODEL],
                        mybir.dt.bfloat16, kind="ExternalInput")
    b1 = nc.dram_tensor("b1", [P, N_EXPERTS * FL], mybir.dt.float32,
                        kind="ExternalInput")
    cw = nc.dram_tensor("cw", [P, NMT], mybir.dt.float32,
                        kind="ExternalInput")
    y = nc.dram_tensor("y", [P, NMT * D_MODEL], mybir.dt.bfloat16,
                       kind="ExternalOutput")

    xT_ap = xT.ap().rearrange("p (ko t) -> p ko t", ko=KO1)
    w1_ap = w1.ap().rearrange("p (e fl ko f) -> p e fl ko f",
                              e=N_EXPERTS, fl=FL, ko=KO1)
    w2_ap = w2.ap().rearrange("p (e fl d) -> p e fl d", e=N_EXPERTS, fl=FL)
    y_ap = y.ap()

    with tile.TileContext(nc) as tc, ExitStack() as ctx:
        wpool = ctx.enter_context(tc.tile_pool(name="wpool", bufs=1))
        xpool = ctx.enter_context(tc.tile_pool(name="xpool", bufs=4))
        hpool = ctx.enter_context(tc.tile_pool(name="hpool", bufs=1))
        ypool = ctx.enter_context(tc.tile_pool(name="ypool", bufs=3))
        ps1 = ctx.enter_context(tc.tile_pool(name="ps1", bufs=2, space="PSUM"))
        ps2 = ctx.enter_context(tc.tile_pool(name="ps2", bufs=5, space="PSUM"))

        for rep in range(reps):
            b1s = wpool.tile([P, N_EXPERTS * FL], mybir.dt.float32,
                             tag="b1s", name="b1s")
            cws = wpool.tile([P, NMT], mybir.dt.float32, tag="cws",
                             name="cws")
            w1s = wpool.tile([P, N_EXPERTS, FL, KO1, P], mybir.dt.bfloat16,
                             tag="w1s", name="w1s")
            w2s = wpool.tile([P, N_EXPERTS, FL, D_MODEL], mybir.dt.bfloat16,
                             tag="w2s", name="w2s")

            def load_x(gi):
                e, moff, mt = groups[gi]
                xt = xpool.tile([P, KO1, 512], mybir.dt.bfloat16, tag="xg",
                                name=f"x_{gi}")
                c0 = moff * P
                nc.gpsimd.dma_start(xt[:, :, :mt * P],
                                    xT_ap[:, :, c0:c0 + mt * P])
                return xt

            # Queue plan: gpsimd carries x loads only, sync carries y
            # writebacks only, and weights ride the (otherwise idle) scalar
            # queue just-in-time — front-loading 16 MiB of weights onto the
            # x/y queues head-of-line-blocks the pipeline for ~70 us.
            #
            # Ramp-critical DMAs first: w1 slice of the first expert in 4
            # per-fl chunks on sync (first mm1 chain starts after chunk 0),
            # x group 0 + b1 on gpsimd.
            e0 = groups[0][0]
            for fl in range(FL):
                nc.sync.dma_start(w1s[:, e0, fl], w1_ap[:, e0, fl])
            x_pending = {0: load_x(0)}
            nc.gpsimd.dma_start(b1s[:], b1.ap())
            for gi in range(1, min(3, NG)):
                x_pending[gi] = load_x(gi)
            nc.gpsimd.dma_start(cws[:], cw.ap())

            # Weight prefetch order (scalar queue): w2 of the current
            # expert, then w1/w2 of the next, issued at expert boundaries
            # as the group loop runs (see below).
            used = sorted(set(e for e, _, _ in groups))
            nxt = {e: used[j + 1] if j + 1 < len(used) else None
                   for j, e in enumerate(used)}
            nc.scalar.dma_start(w2s[:, e0], w2_ap[:, e0])
            if nxt[e0] is not None:
                nc.scalar.dma_start(w1s[:, nxt[e0]], w1_ap[:, nxt[e0]])
                nc.scalar.dma_start(w2s[:, nxt[e0]], w2_ap[:, nxt[e0]])

            def h_tile(fl, par):
                return hpool.tile([P, 512], mybir.dt.bfloat16,
                                  tag=f"h_{fl}_{par}", name=f"h_{fl}_{par}")

            def emit_mm1(gi, fl, xt, h_cur):
                e, moff, mt = groups[gi]
                gtb = mt * P
                pt = ps1.tile([P, 512], mybir.dt.float32, tag="ps1",
                              name="pt1")
                for ko in range(KO1):
                    nc.tensor.matmul(
                        pt[:, :gtb],
                        w1s[:, e, fl, ko, :],
                        xt[:, ko, :gtb],
                        start=(ko == 0),
                        stop=(ko == KO1 - 1),
                    )
                hf = h_tile(fl, gi % 2)
                nc.scalar.activation(
                    hf[:, :gtb], pt[:, :gtb],
                    mybir.ActivationFunctionType.Relu,
                    bias=b1s[:, e * FL + fl:e * FL + fl + 1],
                )
                h_cur.append(hf)

            def emit_chain(gi, m, half, h_prev, ytile):
                e, moff, mt = groups[gi]
                pt2 = ps2.tile([P, 512], mybir.dt.float32, tag="ps2",
                               name="pt2")
                for fl in range(FL):
                    nc.tensor.matmul(
                        pt2[:],
                        h_prev[fl][:, m * P:(m + 1) * P],
                        w2s[:, e, fl, half * 512:(half + 1) * 512],
                        start=(fl == 0),
                        stop=(fl == FL - 1),
                    )
                nc.vector.tensor_scalar_mul(
                    ytile[:, m * D_MODEL + half * 512:
                          m * D_MODEL + (half + 1) * 512],
                    pt2[:],
                    cws[:, moff + m:moff + m + 1],
                )

            h_prev = None
            for gi in range(NG + 1):
                cur = gi if gi < NG else None
                prev = gi - 1 if gi > 0 else None

                if cur is not None and gi + 3 < NG:
                    x_pending[gi + 3] = load_x(gi + 3)
                # At each expert's first group, prefetch the following
                # expert's weights (a full expert-span of lead time).
                if cur is not None and gi > 0:
                    e_cur = groups[gi][0]
                    if e_cur != groups[gi - 1][0]:
                        ne = nxt[e_cur]
                        if ne is not None:
                            nc.scalar.dma_start(w1s[:, ne], w1_ap[:, ne])
                            nc.scalar.dma_start(w2s[:, ne], w2_ap[:, ne])

                if prev is not None:
                    pe, pmoff, pmt = groups[prev]
                    chains = [(m, h) for m in range(pmt) for h in range(2)]
                else:
                    chains = []

                if cur is None:
                    # Final drain: per-chain psum -> scale -> dma so the
                    # writeback overlaps the remaining chains' matmuls.
                    for m, half in chains:
                        pt2 = ps2.tile([P, 512], mybir.dt.float32,
                                       tag="ps2", name="pt2")
                        for fl in range(FL):
                            nc.tensor.matmul(
                                pt2[:],
                                h_prev[fl][:, m * P:(m + 1) * P],
                                w2s[:, pe, fl, half * 512:(half + 1) * 512],
                                start=(fl == 0),
                                stop=(fl == FL - 1),
                            )
                        yt = ypool.tile([P, 512], mybir.dt.bfloat16,
                                        tag="ytail", name="ytail")
                        nc.vector.tensor_scalar_mul(
                            yt[:], pt2[:],
                            cws[:, pmoff + m:pmoff + m + 1],
                        )
                        nc.sync.dma_start(
                            y_ap[:, (pmoff + m) * D_MODEL + half * 512:
                                 (pmoff + m) * D_MODEL + (half + 1) * 512],
                            yt[:],
                        )
                    continue

                e, moff, mt = groups[cur]
                xt = x_pending.pop(cur)
                h_cur = []
                ytile = None
                if prev is not None:
                    ytile = ypool.tile([P, pmt * D_MODEL],
                                       mybir.dt.bfloat16, tag="yt",
                                       name="yt")
                    # Distribute prev's 2*pmt mm2 chains over the 4 fl
                    # iterations of cur's mm1.
                    per = [chains[(len(chains) * fl) // FL:
                                  (len(chains) * (fl + 1)) // FL]
                           for fl in range(FL)]
                else:
                    per = [[] for _ in range(FL)]

                for fl in range(FL):
                    emit_mm1(cur, fl, xt, h_cur)
                    for m, half in per[fl]:
                        emit_chain(prev, m, half, h_prev, ytile)
                if prev is not None:
                    nc.sync.dma_start(
                        y_ap[:, pmoff * D_MODEL:(pmoff + pmt) * D_MODEL],
                        ytile[:, :pmt * D_MODEL],
                    )
                h_prev = h_cur

    nc.compile()
    return nc


def _route(x_flat, Wg, bg):
    logits = x_flat.astype(np.float32) @ Wg.astype(np.float32) + bg
    idx = np.argsort(-logits, axis=1, kind="stable")[:, :TOP_K]
    gates = np.take_along_axis(logits, idx, axis=1)  # [T, 2] descending
    e1 = np.exp(gates[:, 1] - gates[:, 0])
    denom = 1.0 + e1
    w = np.stack([1.0 / denom, e1 / denom], axis=1).astype(np.float32)
    return idx.astype(np.int32), w


def _emulate_run(in_maps, spec):
    """Numpy emulation of the device kernel (for layout validation)."""
    NMT = sum(spec)
    TP = NMT * P
    results = []
    for m in in_maps:
        xT = np.asarray(m["xT"], np.float32).reshape(P, KO1, TP)
        w1 = np.asarray(m["w1"], np.float32).reshape(P, N_EXPERTS, FL,
                                                     KO1, P)
        w2 = np.asarray(m["w2"], np.float32).reshape(P, N_EXPERTS, FL,
                                                     D_MODEL)
        b1 = np.asarray(m["b1"], np.float32).reshape(P, N_EXPERTS, FL)
        cwt = np.asarray(m["cw"], np.float32)  # [P, NMT]
        xs = np.ascontiguousarray(xT.transpose(2, 1, 0)).reshape(TP,
                                                                 D_MODEL)
        cwf = np.ascontiguousarray(cwt.T).reshape(TP)
        yv = np.zeros((TP, D_MODEL), np.float32)
        off = 0
        for e, me in enumerate(spec):
            n = me * P
            seg = slice(off, off + n)
            W1e = np.ascontiguousarray(
                w1[:, e].transpose(2, 0, 1, 3)).reshape(D_MODEL, FSL)
            W2e = np.ascontiguousarray(
                w2[:, e].transpose(1, 0, 2)).reshape(FSL, D_MODEL)
            b1e = np.ascontiguousarray(b1[:, e].T).reshape(FSL)
            h = np.maximum(xs[seg] @ W1e + b1e, 0.0)
            h = h.astype(BF16).astype(np.float32)
            yv[seg] = (h @ W2e) * cwf[seg, None]
            off += n
        y_dev = np.ascontiguousarray(
            yv.reshape(NMT, P, D_MODEL).transpose(1, 0, 2)
        ).reshape(P, NMT * D_MODEL).astype(BF16)
        results.append({"y": y_dev})

    class Res:
        pass

    r = Res()
    r.results = results
    r.exec_time_ns = None
    return r


def kernel(x, Wg, bg, W1, b1, W2, b2):
    global LAST_RESULTS, LAST_IN_MAPS, LAST_C
    x = np.asarray(x, dtype=np.float32)
    Wg = np.asarray(Wg, dtype=np.float32)
    bg = np.asarray(bg, dtype=np.float32)
    W1 = np.asarray(W1, dtype=np.float32)
    b1 = np.asarray(b1, dtype=np.float32)
    W2 = np.asarray(W2, dtype=np.float32)
    b2 = np.asarray(b2, dtype=np.float32)

    x_flat = x.reshape(T, D_MODEL)
    idx, w = _route(x_flat, Wg, bg)

    # Per-expert token lists; stream offset + slot map (position of each
    # (token, k) pair inside the padded concatenated stream).
    tok_lists = []
    spec = []
    offs = []
    off = 0
    slot = np.empty((T, TOP_K), dtype=np.int64)
    for e in range(N_EXPERTS):
        mask = (idx[:, 0] == e) | (idx[:, 1] == e)
        tok = np.nonzero(mask)[0]
        tok_lists.append(tok)
        me = (len(tok) + P - 1) // P
        spec.append(me)
        offs.append(off)
        which = (idx[tok, 1] == e).astype(np.int64)  # 0 if k=0 slot else 1
        slot[tok, which] = off + np.arange(len(tok))
        off += me * P
    spec = tuple(spec)
    TP = off
    NMT = TP // P

    # Build the shared token stream + per-core weight slices.
    xs = np.zeros((TP, D_MODEL), dtype=np.float32)
    cw_stream = np.zeros((TP,), dtype=np.float32)
    for e in range(N_EXPERTS):
        tok = tok_lists[e]
        n = len(tok)
        o = offs[e]
        xs[o:o + n] = x_flat[tok]
        cw_stream[o:o + n] = np.where(idx[tok, 0] == e, w[tok, 0],
                                      w[tok, 1])

    xT_dev = np.ascontiguousarray(
        xs.reshape(TP, KO1, P).transpose(2, 1, 0)
    ).reshape(P, KO1 * TP).astype(BF16)
    cw_dev = np.ascontiguousarray(
        cw_stream.reshape(NMT, P).T).astype(np.float32)

    in_maps = []
    for c in range(N_CORES):
        fs = slice(c * FSL, (c + 1) * FSL)
        w1c = np.ascontiguousarray(
            W1[:, :, fs].reshape(N_EXPERTS, KO1, P, FL, P)
            .transpose(2, 0, 3, 1, 4)
        ).reshape(P, N_EXPERTS * FL * KO1 * P).astype(BF16)
        w2c = np.ascontiguousarray(
            W2[:, fs, :].reshape(N_EXPERTS, FL, P, D_MODEL)
            .transpose(2, 0, 1, 3)
        ).reshape(P, N_EXPERTS * FL * D_MODEL).astype(BF16)
        b1c = np.ascontiguousarray(
            b1[:, fs].reshape(N_EXPERTS, FL, P).transpose(2, 0, 1)
        ).reshape(P, N_EXPERTS * FL).astype(np.float32)
        in_maps.append({
            "xT": xT_dev,
            "w1": w1c,
            "w2": w2c,
            "b1": b1c,
            "cw": cw_dev,
        })

    LAST_IN_MAPS = in_maps
    LAST_C = spec

    if os.environ.get("MOE_KERNEL_EMULATE", "0") == "1":
        res = _emulate_run(in_maps, spec)
    else:
        if spec not in _NC_CACHE:
            _NC_CACHE[spec] = _build_nc(spec)
        nc = _NC_CACHE[spec]

        from concourse.bass_utils import run_bass_kernel_spmd

        trace = os.environ.get("MOE_KERNEL_TRACE", "0") == "1"
        res = run_bass_kernel_spmd(
            nc, in_maps, core_ids=list(range(N_CORES)),
            trace=trace, trace_cores=[0] if trace else None,
        )
    LAST_RESULTS = res

    # Sum the 8 partial y streams, then combine per token.
    Ysum = np.zeros((TP, D_MODEL), dtype=np.float32)
    for c in range(N_CORES):
        y_dev = np.asarray(res.results[c]["y"], dtype=np.float32)
        Ysum += y_dev.reshape(P, NMT, D_MODEL).transpose(1, 0, 2).reshape(
            TP, D_MODEL)

    out_flat = Ysum[slot[:, 0]] + Ysum[slot[:, 1]]

    if np.any(b2):
        out_flat += w[:, 0:1] * b2[idx[:, 0]] + w[:, 1:2] * b2[idx[:, 1]]

    return out_flat.reshape(B, S, D_MODEL).astype(np.float32)
